# revision 1
# baseline (speedup 1.0000x reference)
"""Multi-head attention (B=2, S=2048, D=1024, H=16, HD=64) on 8 TRN2 cores.

Sharding (hybrid DP/TP, SPMD one-graph):
  core c: batch b = c//4, head-group g = c%4 (heads 4g..4g+3 of batch b).
  - QKV projections: Megatron column-split (each core its 4 heads).
  - attention: fully local per (batch, head).
  - att outputs (attT layout [hd, S] per head, bf16) AllGather'd per head
    within the 4-core batch group -> every core holds all 16 heads.
  - O-projection: Megatron column-split on wo (each core owns 256 output
    channels for ALL tokens of its batch; the wo column slice is a
    per-core input, so the compiled graph is identical across cores).
  - host gather: pure concat over (batch, output-channel slice).

Layouts on chip:
  xT  [D, S]  (x transposed on host)  -> SBUF [128, 8, 2048] f32r
  qT/kT [heads*hd, S] -> SBUF [128, 2, 2048] f32r (computed via PE)
  v natural [S, 4 heads, 128] bf16 where cols = [v(64) | ones | zeros]
  scoresT [s_k part, s_q free] in PSUM; exp on ACT -> bf16; PV matmul
  -> psum [128, 512] whose row 64 is the softmax denominator.
QK/scores matmuls run as float32r (fp32 storage, 1 cycle/row on PE);
the exp/PV/O-proj path runs bf16 (f32 PSUM accumulation).
"""

import numpy as np
import ml_dtypes

B, S, D = 2, 2048, 1024
H, HD = 16, 64
N_CORES = 8
G = 4                      # cores per batch group
HPC = 4                    # heads per core
CW = HPC * HD              # per-core projection width = 256
ATT_SCALE = float(HD) ** -0.5
P = 128

_CACHED_NC = None


def _build():
    import concourse.mybir as mybir
    import concourse.tile as tile
    from concourse import bacc

    f32 = mybir.dt.float32
    f32r = mybir.dt.float32r
    bf16 = mybir.dt.bfloat16
    Exp = mybir.ActivationFunctionType.Exp
    add = mybir.AluOpType.add
    mult = mybir.AluOpType.mult

    nc = bacc.Bacc("TRN2", target_bir_lowering=False, debug=False,
                   num_devices=N_CORES)

    xT = nc.declare_dram_parameter("xT", [D, S], f32r, isOutput=False)
    wq = nc.declare_dram_parameter("wq", [D, CW], f32r, isOutput=False)
    wk = nc.declare_dram_parameter("wk", [D, CW], f32r, isOutput=False)
    wv = nc.declare_dram_parameter("wv", [D, CW], f32r, isOutput=False)
    bq = nc.declare_dram_parameter("bq", [CW], f32, isOutput=False)
    bk = nc.declare_dram_parameter("bk", [CW], f32, isOutput=False)
    bv = nc.declare_dram_parameter("bv", [CW], f32, isOutput=False)
    wo = nc.declare_dram_parameter("wo", [D, CW], bf16, isOutput=False)
    bo = nc.declare_dram_parameter("bo", [CW], f32, isOutput=False)
    out = nc.declare_dram_parameter("out", [S, CW], f32, isOutput=True)

    groups = [[0, 1, 2, 3], [4, 5, 6, 7]]
    KC = D // P           # 8 contraction chunks
    SC = S // P           # 16 token chunks of 128
    NQ = 512              # moving free dim per matmul
    SQC = S // NQ         # 4 query chunks of 512

    with tile.TileContext(nc) as tc:
        with (
            tc.tile_pool(name="const", bufs=1) as const,
            tc.tile_pool(name="acts", bufs=1) as acts,
            tc.tile_pool(name="exps", bufs=6) as exps,
            tc.tile_pool(name="attw", bufs=4) as attw,
            tc.tile_pool(name="small", bufs=2) as small,
            tc.tile_pool(name="ostage", bufs=3) as ostage,
            tc.tile_pool(name="dram", bufs=1, space="DRAM") as dram,
        ):
            # ---- constant loads -------------------------------------
            xt_sb = const.tile([P, KC, S], f32r)
            for ki in range(KC):
                nc.sync.dma_start(xt_sb[:, ki, :],
                                  xT[ki * P:(ki + 1) * P, :])
            wq_sb = const.tile([P, KC, CW], f32r, tag="wq")
            wk_sb = const.tile([P, KC, CW], f32r, tag="wk")
            wv_sb = const.tile([P, KC, CW], f32r, tag="wv")
            nc.sync.dma_start(wq_sb[:], wq.ap().rearrange("(k p) m -> p k m", p=P))
            nc.sync.dma_start(wk_sb[:], wk.ap().rearrange("(k p) m -> p k m", p=P))
            nc.sync.dma_start(wv_sb[:], wv.ap().rearrange("(k p) m -> p k m", p=P))
            wo_sb = const.tile([P, KC, CW], bf16, tag="wo")
            nc.sync.dma_start(wo_sb[:], wo.ap().rearrange("(k p) n -> p k n", p=P))
            bq_sb = const.tile([P, 2], f32, tag="bq")
            bk_sb = const.tile([P, 2], f32, tag="bk")
            nc.sync.dma_start(bq_sb[:], bq.ap().rearrange("(j p) -> p j", p=P))
            nc.sync.dma_start(bk_sb[:], bk.ap().rearrange("(j p) -> p j", p=P))
            bv_bc = const.tile([P, CW], f32, tag="bv")
            bo_bc = const.tile([P, CW], f32, tag="bo")
            nc.sync.dma_start(bv_bc[:], bv.ap().partition_broadcast(P))
            nc.sync.dma_start(bo_bc[:], bo.ap().partition_broadcast(P))

            ones_f = const.tile([1, HD], f32, tag="onesf")
            ones_r = const.tile([1, HD], f32r, tag="onesr")
            nc.vector.memset(ones_f[:], 1.0)
            with nc.allow_low_precision("f32r is fp32 storage"):
                nc.vector.tensor_copy(ones_r[:], ones_f[:])

            qT_sb = acts.tile([P, 2, S], f32r, tag="qT")
            kT_sb = acts.tile([P, 2, S], f32r, tag="kT")
            # v cols per head: [v(64) | ones(1) | zeros(63)] -> lhsT M=128
            v_sb = acts.tile([P, SC, HPC, P], bf16, tag="v")
            nc.vector.memset(v_sb[:, :, :, HD + 1:], 0.0)
            nc.vector.memset(v_sb[:, :, :, HD:HD + 1], 1.0)

            # ---- projections ----------------------------------------
            with tc.tile_pool(name="pp", bufs=3, space="PSUM") as pp:
                # qT / kT: [128(2 heads*hd), S] = w_chunk.T @ xT
                for (w_sb, b_sb, dst) in ((wq_sb, bq_sb, qT_sb),
                                          (wk_sb, bk_sb, kT_sb)):
                    for j in range(2):
                        for si in range(SQC):
                            ps = pp.tile([P, NQ], f32, tag="pq")
                            for ki in range(KC):
                                nc.tensor.matmul(
                                    ps[:],
                                    w_sb[:, ki, j * P:(j + 1) * P],
                                    xt_sb[:, ki, si * NQ:(si + 1) * NQ],
                                    start=(ki == 0), stop=(ki == KC - 1),
                                )
                            with nc.allow_low_precision("f32r is fp32 storage"):
                                nc.vector.tensor_tensor(
                                    dst[:, j, si * NQ:(si + 1) * NQ], ps[:],
                                    b_sb[:, j:j + 1].to_broadcast((P, NQ)),
                                    add)
                # v natural: [S, 256] = xT_chunk.T @ wv
                for si in range(SC):
                    ps = pp.tile([P, CW], f32, tag="pv")
                    for ki in range(KC):
                        nc.tensor.matmul(
                            ps[:],
                            xt_sb[:, ki, si * P:(si + 1) * P],
                            wv_sb[:, ki, :],
                            start=(ki == 0), stop=(ki == KC - 1),
                        )
                    nc.vector.tensor_tensor(
                        v_sb[:, si, :, :HD],
                        ps.rearrange("p (h x) -> p h x", x=HD),
                        bv_bc.rearrange("p (h x) -> p h x", x=HD), add)

            # ---- attention + per-head AllGather ---------------------
            agin = [dram.tile([HD, S], bf16, tag=f"agin{h}", name=f"agin{h}")
                    for h in range(HPC)]
            agout = [dram.tile([G, HD, S], bf16, tag=f"agout{h}",
                               name=f"agout{h}") for h in range(HPC)]
            GRP = 4  # mi-chunks per batched run (uniform PE runs of 8)
            with (
                tc.tile_pool(name="sc", bufs=3, space="PSUM") as scp,
                tc.tile_pool(name="pv", bufs=2, space="PSUM") as pvp,
            ):
                for h in range(HPC):
                    j2, off = h // 2, (h % 2) * HD
                    for half in range(2):
                        pv_ps = [pvp.tile([P, NQ], f32, tag="pv",
                                          name=f"pv{h}_{half}_{i}")
                                 for i in range(2)]
                        for g0 in range(0, SC, GRP):
                            sct = []
                            for mi in range(g0, g0 + GRP):
                                sc_ps = scp.tile([P, 2 * NQ], f32, tag="sc",
                                                 name=f"sc{h}_{half}_{mi}")
                                sct.append(sc_ps)
                                for qq in range(2):
                                    sq = half * 2 + qq
                                    nc.tensor.matmul(
                                        sc_ps[:, qq * NQ:(qq + 1) * NQ],
                                        kT_sb[off:off + HD, j2,
                                              mi * P:(mi + 1) * P],
                                        qT_sb[off:off + HD, j2,
                                              sq * NQ:(sq + 1) * NQ],
                                        start=True, stop=True,
                                    )
                            ets = []
                            for i, mi in enumerate(range(g0, g0 + GRP)):
                                et = exps.tile([P, 2 * NQ], bf16, tag="exp",
                                               name=f"et{h}_{half}_{mi}")
                                ets.append(et)
                                nc.scalar.activation(et[:], sct[i][:], Exp,
                                                     scale=ATT_SCALE)
                            for i, mi in enumerate(range(g0, g0 + GRP)):
                                for qq in range(2):
                                    nc.tensor.matmul(
                                        pv_ps[qq][:],
                                        v_sb[:, mi, h, :],
                                        ets[i][:, qq * NQ:(qq + 1) * NQ],
                                        start=(mi == 0), stop=(mi == SC - 1),
                                    )
                        # softmax divide + bf16 cast, stage for AllGather
                        for qq in range(2):
                            sq = half * 2 + qq
                            rec = small.tile([1, NQ], f32r, tag="rec")
                            with nc.allow_low_precision("f32r is fp32"):
                                nc.vector.reciprocal(
                                    rec[:], pv_ps[qq][HD:HD + 1, :])
                            rb = scp.tile([HD, NQ], f32, tag="sc",
                                          name=f"rb{h}_{sq}")
                            nc.tensor.matmul(rb[:], ones_r[:], rec[:],
                                             start=True, stop=True)
                            rb_sb = attw.tile([HD, NQ], f32, tag="rbs")
                            nc.vector.tensor_copy(rb_sb[:], rb[:])
                            at = attw.tile([HD, NQ], bf16, tag="att")
                            nc.vector.tensor_tensor(
                                at[:], pv_ps[qq][:HD, :], rb_sb[:], mult)
                            nc.sync.dma_start(
                                agin[h][:, sq * NQ:(sq + 1) * NQ], at[:])
                    nc.gpsimd.collective_compute(
                        "AllGather", mybir.AluOpType.bypass,
                        replica_groups=groups,
                        ins=[agin[h].opt()],
                        outs=[agout[h].opt()],
                    )

            # ---- O-projection (column-sharded, all tokens) ----------
            # kc-outer so each AllGather's contribution starts as soon as it
            # lands; 16 accumulators packed 2-per-PSUM-bank.
            with (
                tc.tile_pool(name="attk", bufs=3) as attk,
                tc.tile_pool(name="op", bufs=8, space="PSUM") as op,
            ):
                for hs in range(2):  # two passes of 8 token-chunks each
                    po = [op.tile([P, CW], f32, tag="po",
                                  name=f"po{hs}_{i}") for i in range(8)]
                    for kc in range(KC):
                        h, jj = kc // 2, kc % 2
                        atk = attk.tile([P, S // 2], bf16, tag="atk",
                                        name=f"atk{hs}_{kc}")
                        nc.sync.dma_start(
                            atk[:],
                            agout[h][2 * jj:2 * jj + 2, :,
                                     hs * (S // 2):(hs + 1) * (S // 2)]
                            .rearrange("a b s -> (a b) s"))
                        for i in range(8):
                            nc.tensor.matmul(
                                po[i][:],
                                atk[:, i * P:(i + 1) * P],
                                wo_sb[:, kc, :],
                                start=(kc == 0), stop=(kc == KC - 1),
                            )
                    for i in range(8):
                        si = hs * 8 + i
                        ot = ostage.tile([P, CW], f32, tag="ot")
                        nc.vector.tensor_tensor(ot[:], po[i][:], bo_bc[:],
                                                add)
                        nc.sync.dma_start(out[si * P:(si + 1) * P, :], ot[:])

    nc.compile()
    return nc


def _get_nc():
    global _CACHED_NC
    if _CACHED_NC is None:
        _CACHED_NC = _build()
    return _CACHED_NC


# permutation of global head index by (ag_h, source core j): head 4j+h
_HEAD_ORDER = [4 * j + h for h in range(HPC) for j in range(G)]


def kernel(x, wq, bq, wk, bk, wv, bv, wo, bo):
    from concourse.bass_utils import run_bass_kernel_spmd

    x = np.asarray(x, dtype=np.float32)
    wq = np.asarray(wq, dtype=np.float32)
    wk = np.asarray(wk, dtype=np.float32)
    wv = np.asarray(wv, dtype=np.float32)
    wo = np.asarray(wo, dtype=np.float32)
    bq = np.asarray(bq, dtype=np.float32)
    bk = np.asarray(bk, dtype=np.float32)
    bv = np.asarray(bv, dtype=np.float32)
    bo = np.asarray(bo, dtype=np.float32)

    nc = _get_nc()

    # wo rows reordered to the (ag_h, source_core) K-chunk order used on chip
    wo_perm = np.ascontiguousarray(
        wo.reshape(H, HD, D)[_HEAD_ORDER].reshape(D, D))

    in_maps = []
    for c in range(N_CORES):
        b, g = c // G, c % G
        cs = slice(g * CW, (g + 1) * CW)
        in_maps.append({
            "xT": np.ascontiguousarray(x[b].T),
            "wq": np.ascontiguousarray(wq[:, cs]),
            "wk": np.ascontiguousarray(wk[:, cs]),
            "wv": np.ascontiguousarray(wv[:, cs]),
            "bq": np.ascontiguousarray(bq[cs]),
            "bk": np.ascontiguousarray(bk[cs]),
            "bv": np.ascontiguousarray(bv[cs]),
            "wo": np.ascontiguousarray(wo_perm[:, cs]).astype(
                ml_dtypes.bfloat16),
            "bo": np.ascontiguousarray(bo[cs]),
        })

    res = run_bass_kernel_spmd(nc, in_maps, core_ids=list(range(N_CORES)))

    full = np.empty((B, S, D), dtype=np.float32)
    for c in range(N_CORES):
        b, g = c // G, c % G
        full[b, :, g * CW:(g + 1) * CW] = res.results[c]["out"]
    return full



# revision 20
# speedup vs baseline: 1.3944x; 1.3944x over previous
"""Multi-head attention (B=2, S=2048, D=1024, H=16, HD=64) on 8 TRN2 cores.

Sharding (hybrid DP/TP, SPMD one-graph):
  core c: batch b = c//4, head-group g = c%4 (heads 4g..4g+3 of batch b).
  - QKV projections: Megatron column-split (each core its 4 heads), bf16.
  - queries are token-quarter XOR-permuted per core (local quarter s =
    true quarter s^g) via a separate host-permuted xqT input, so the
    exchange below is SPMD-uniform.
  - attention per (batch, head): QK^T row-packed 2 heads/pass
    (tile_position (0,0)/(64,0)); softmax exp split 40% ACT (exact) /
    60% DVE (Schraudolph int16-bitcast bf16); PV bf16 with a ones
    column producing the softmax denominator in row 64.
  - exchange: 6 pairwise (2-rank) AllGathers per core: round m pairs
    core g with g^m; both send their heads' att for the partner's true
    token quarter (local quarter m).  Wire: 3/4 of att, bf16.
  - O-projection: token-sharded — each core computes its true quarter
    (512 tokens) x full D with per-core-arranged wo; partner half of
    each AllGather selected with a per-core 0/1 input (DVE blend).
  - host gather: concat over (batch, token quarter).
"""

import numpy as np
import ml_dtypes

B, S, D = 2, 2048, 1024
H, HD = 16, 64
N_CORES = 8
G = 4                      # cores per batch group
HPC = 4                    # heads per core
CW = HPC * HD              # per-core projection width = 256
ATT_SCALE = float(HD) ** -0.5
P = 128
KC = D // P                # 8 contraction chunks
SC = S // P                # 16 key chunks of 128
NQ = 512                   # query chunk (= token quarter)
SQC = S // NQ              # 4 query chunks

LOG2E = 1.4426950408889634
# Schraudolph exp via int16 bitcast to bf16: bits = round(A16*x + B16)
A16 = 128.0 * LOG2E * ATT_SCALE
B16 = 128.0 * 127 - 7.33

# round-m 2-rank pairings (g <-> g^m) within each batch group
RG = {
    1: [[0, 1], [2, 3], [4, 5], [6, 7]],
    2: [[0, 2], [1, 3], [4, 6], [5, 7]],
    3: [[0, 3], [1, 2], [4, 7], [5, 6]],
}

_CACHED_NC = None


def _build():
    import concourse.mybir as mybir
    import concourse.tile as tile
    from concourse import bacc

    f32 = mybir.dt.float32
    f32r = mybir.dt.float32r
    bf16 = mybir.dt.bfloat16
    i16 = mybir.dt.int16
    Exp = mybir.ActivationFunctionType.Exp
    add = mybir.AluOpType.add
    mult = mybir.AluOpType.mult

    nc = bacc.Bacc("TRN2", target_bir_lowering=False, debug=False,
                   num_devices=N_CORES)

    xT = nc.declare_dram_parameter("xT", [D, S], bf16, isOutput=False)
    xqT = nc.declare_dram_parameter("xqT", [D, S], bf16, isOutput=False)
    wq = nc.declare_dram_parameter("wq", [D, CW], bf16, isOutput=False)
    wk = nc.declare_dram_parameter("wk", [D, CW], bf16, isOutput=False)
    wv = nc.declare_dram_parameter("wv", [D, CW], bf16, isOutput=False)
    bq = nc.declare_dram_parameter("bq", [CW], f32, isOutput=False)
    bk = nc.declare_dram_parameter("bk", [CW], f32, isOutput=False)
    bv = nc.declare_dram_parameter("bv", [CW], f32, isOutput=False)
    # wo pre-arranged per core: [ki=(l,hd), chunk=2m+pair, n]
    wo = nc.declare_dram_parameter("wo", [P, 8, D], bf16, isOutput=False)
    bo = nc.declare_dram_parameter("bo", [D], f32, isOutput=False)
    sel = nc.declare_dram_parameter("sel", [P, 3, 2], f32, isOutput=False)
    out = nc.declare_dram_parameter("out", [NQ, D], f32, isOutput=True)

    with tile.TileContext(nc) as tc:
        with (
            tc.tile_pool(name="const", bufs=1) as const,
            tc.tile_pool(name="acts", bufs=1) as acts,
            tc.tile_pool(name="exps", bufs=4) as exps,
            tc.tile_pool(name="small", bufs=2) as small,
            tc.tile_pool(name="ostage", bufs=3) as ostage,
            tc.tile_pool(name="dram", bufs=1, space="DRAM") as dram,
        ):
            # ---- constant loads -------------------------------------
            wq_sb = const.tile([P, KC, CW], bf16, tag="wq")
            wk_sb = const.tile([P, KC, CW], bf16, tag="wk")
            wv_sb = const.tile([P, KC, CW], bf16, tag="wv")
            nc.sync.dma_start(wq_sb[:], wq.ap().rearrange("(k p) m -> p k m", p=P))
            nc.sync.dma_start(wk_sb[:], wk.ap().rearrange("(k p) m -> p k m", p=P))
            xqt_sb = const.tile([P, KC, S], bf16, tag="xq")
            xt_sb = const.tile([P, KC, S], bf16, tag="x")
            for ki in range(KC):
                nc.sync.dma_start(xqt_sb[:, ki, :], xqT[ki * P:(ki + 1) * P, :])
            for ki in range(KC):
                nc.sync.dma_start(xt_sb[:, ki, :], xT[ki * P:(ki + 1) * P, :])
            nc.sync.dma_start(wv_sb[:], wv.ap().rearrange("(k p) m -> p k m", p=P))
            wo_sb = const.tile([P, 8, D], bf16, tag="wo")
            nc.sync.dma_start(wo_sb[:], wo.ap())
            bq_sb = const.tile([P, 2], f32, tag="bq")
            bk_sb = const.tile([P, 2], f32, tag="bk")
            nc.sync.dma_start(bq_sb[:], bq.ap().rearrange("(j p) -> p j", p=P))
            nc.sync.dma_start(bk_sb[:], bk.ap().rearrange("(j p) -> p j", p=P))
            bv_bc = const.tile([P, CW], f32, tag="bv")
            bo_bc = const.tile([P, D], f32, tag="bo")
            nc.sync.dma_start(bv_bc[:], bv.ap().partition_broadcast(P))
            nc.sync.dma_start(bo_bc[:], bo.ap().partition_broadcast(P))
            sel_sb = const.tile([P, 3, 2], f32, tag="sel")
            nc.sync.dma_start(sel_sb[:], sel.ap())

            ones_f = const.tile([1, HD], f32, tag="onesf")
            ones_r = const.tile([1, HD], f32r, tag="onesr")
            nc.vector.memset(ones_f[:], 1.0)
            with nc.allow_low_precision("f32r is fp32 storage"):
                nc.vector.tensor_copy(ones_r[:], ones_f[:])

            qT_sb = acts.tile([P, 2, S], bf16, tag="qT")
            kT_sb = acts.tile([P, 2, S], bf16, tag="kT")
            # v per (mi, head): cols [v(64) | ones]
            v_sb = acts.tile([P, SC, HPC, HD + 1], bf16, tag="v")
            nc.vector.memset(v_sb[:, :, :, HD:HD + 1], 1.0)

            # ---- projections (q pair0, k pair0, v, q pair1, k pair1) -
            with tc.tile_pool(name="pp", bufs=3, space="PSUM") as pp:
                def qk_proj(j):
                    for (w_sb, b_sb, dst, src) in (
                            (wq_sb, bq_sb, qT_sb, xqt_sb),
                            (wk_sb, bk_sb, kT_sb, xt_sb)):
                        for si in range(SQC):
                            ps = pp.tile([P, NQ], f32, tag="pq")
                            for ki in range(KC):
                                nc.tensor.matmul(
                                    ps[:],
                                    w_sb[:, ki, j * P:(j + 1) * P],
                                    src[:, ki, si * NQ:(si + 1) * NQ],
                                    start=(ki == 0), stop=(ki == KC - 1),
                                )
                            with nc.allow_low_precision("bf16 qk"):
                                nc.vector.tensor_tensor(
                                    dst[:, j, si * NQ:(si + 1) * NQ], ps[:],
                                    b_sb[:, j:j + 1].to_broadcast((P, NQ)),
                                    add)

                qk_proj(0)
                for si in range(SC):
                    ps = pp.tile([P, CW], f32, tag="pv")
                    for ki in range(KC):
                        nc.tensor.matmul(
                            ps[:],
                            xt_sb[:, ki, si * P:(si + 1) * P],
                            wv_sb[:, ki, :],
                            start=(ki == 0), stop=(ki == KC - 1),
                        )
                    with nc.allow_low_precision("bf16 v"):
                        nc.vector.tensor_tensor(
                            v_sb[:, si, :, :HD],
                            ps.rearrange("p (h x) -> p h x", x=HD),
                            bv_bc.rearrange("p (h x) -> p h x", x=HD), add)
                qk_proj(1)

            # ---- attention + pairwise exchange ----------------------
            # stg[j2][sq] = my 2 heads' att for local quarter sq
            # (true quarter sq^g); sq=0 stays local, sq=m goes to
            # round-m partner g^m.
            stg = [dram.tile([SQC, P, NQ], bf16, tag=f"stg{p}",
                             name=f"stg{p}") for p in range(2)]
            agout = {}
            for j2 in range(2):
                for m in (1, 2, 3):
                    agout[(j2, m)] = dram.tile(
                        [2, P, NQ], bf16, tag=f"ago{j2}_{m}",
                        name=f"ago{j2}_{m}")
            with (
                tc.tile_pool(name="sc", bufs=3, space="PSUM") as scp,
                tc.tile_pool(name="pv", bufs=2, space="PSUM") as pvp,
            ):
                ucount = 0
                for j2 in range(2):              # head pair
                    for sq in (1, 2, 3, 0):      # own quarter last
                        pvs = [pvp.tile([HD + 1, NQ], f32, tag="pv",
                                        name=f"pv{j2}_{sq}_{l}")
                               for l in range(2)]
                        for mp in range(SC // 2):   # key-chunk pair
                            scs = [scp.tile([P, 2, NQ], f32, tag="sc",
                                            name=f"sc{j2}_{sq}_{mp}_{l}")
                                   for l in range(2)]
                            # QK row-packed: head l at rows 64*l
                            for u in range(2):
                                mi = 2 * mp + u
                                for l in range(2):
                                    o = l * HD
                                    nc.tensor.matmul(
                                        scs[l][:, u, :],
                                        kT_sb[o:o + HD, j2,
                                              mi * P:(mi + 1) * P],
                                        qT_sb[o:o + HD, j2,
                                              sq * NQ:(sq + 1) * NQ],
                                        start=True, stop=True,
                                    )
                            # exp: ~40% ACT (exact), ~60% DVE Schraudolph
                            ets = []
                            for l in range(2):
                                use_act = (ucount % 5) in (0, 3)
                                ucount += 1
                                et = exps.tile([P, 2, NQ], bf16, tag="et")
                                if use_act:
                                    nc.scalar.activation(
                                        et[:], scs[l][:], Exp,
                                        scale=ATT_SCALE)
                                else:
                                    with nc.allow_low_precision(
                                            "schraudolph exp"):
                                        nc.vector.tensor_scalar(
                                            et[:].bitcast(i16), scs[l][:],
                                            A16, B16, mult, add)
                                ets.append(et)
                            for l in range(2):
                                h = 2 * j2 + l
                                for u in range(2):
                                    mi = 2 * mp + u
                                    nc.tensor.matmul(
                                        pvs[l][:],
                                        v_sb[:, mi, h, :],
                                        ets[l][:, u, :],
                                        start=(mi == 0), stop=(mi == SC - 1),
                                    )
                        # softmax divide, stage for exchange
                        for l in range(2):
                            den = small.tile([1, NQ], f32, tag="den")
                            nc.vector.tensor_copy(den[:], pvs[l][HD:HD + 1, :])
                            rec = small.tile([1, NQ], f32, tag="rec")
                            rec_r = small.tile([1, NQ], f32r, tag="recr")
                            with nc.allow_low_precision("approx recip"):
                                nc.vector.reciprocal_approx_fast(
                                    rec[:], den[:])
                                nc.vector.tensor_copy(rec_r[:], rec[:])
                            rb = scp.tile([HD, NQ], f32, tag="sc",
                                          name=f"rb{j2}_{sq}_{l}")
                            nc.tensor.matmul(rb[:], ones_r[:], rec_r[:],
                                             start=True, stop=True)
                            rb_sb = small.tile([HD, NQ], f32, tag="rbs")
                            nc.vector.tensor_copy(rb_sb[:], rb[:])
                            at = small.tile([HD, NQ], bf16, tag="at")
                            with nc.allow_low_precision("bf16 att"):
                                nc.vector.tensor_tensor(
                                    at[:], pvs[l][:HD, :], rb_sb[:], mult)
                            nc.sync.dma_start(
                                stg[j2][sq, l * HD:(l + 1) * HD, :], at[:])
                        if sq in (1, 2, 3):
                            nc.gpsimd.collective_compute(
                                "AllGather", mybir.AluOpType.bypass,
                                replica_groups=RG[sq],
                                ins=[stg[j2][sq, :, :]],
                                outs=[agout[(j2, sq)].opt()],
                            )

            # ---- O-projection (token-sharded, full width) -----------
            with (
                tc.tile_pool(name="attk", bufs=3) as attk,
                tc.tile_pool(name="op", bufs=8, space="PSUM") as op,
            ):
                po = [op.tile([P, NQ], f32, tag="po", name=f"po{i}")
                      for i in range(8)]

                def oproj_chunk(c, atk):
                    for tc_i in range(SQC):
                        for ch in range(2):
                            nc.tensor.matmul(
                                po[tc_i * 2 + ch][:],
                                atk[:, tc_i * P:(tc_i + 1) * P],
                                wo_sb[:, c, ch * NQ:(ch + 1) * NQ],
                                start=(c == 0), stop=(c == 7),
                            )

                for m in range(4):      # 0 = local, 1..3 = rounds
                    for j2 in range(2):
                        c = 2 * m + j2
                        if m == 0:
                            atk = attk.tile([P, NQ], bf16, tag="atk",
                                            name=f"atk{c}")
                            nc.sync.dma_start(atk[:], stg[j2][0, :, :])
                        else:
                            a2 = attk.tile([P, 2, NQ], bf16, tag="a2",
                                           name=f"a2_{c}")
                            nc.sync.dma_start(
                                a2[:],
                                agout[(j2, m)][:, :, :]
                                .rearrange("h r s -> r h s"))
                            t0 = attk.tile([P, NQ], bf16, tag="t0",
                                           name=f"t0_{c}")
                            atk = attk.tile([P, NQ], bf16, tag="atk",
                                            name=f"atk{c}")
                            with nc.allow_low_precision("half select"):
                                nc.vector.tensor_scalar(
                                    t0[:], a2[:, 0, :],
                                    sel_sb[:, m - 1, 0:1], None, mult)
                                nc.vector.tensor_scalar(
                                    atk[:], a2[:, 1, :],
                                    sel_sb[:, m - 1, 1:2], None, mult)
                                nc.vector.tensor_tensor(
                                    atk[:], atk[:], t0[:], add)
                        oproj_chunk(c, atk)

                for tc_i in range(SQC):
                    for ch in range(2):
                        ot = ostage.tile([P, NQ], f32, tag="ot")
                        nc.vector.tensor_tensor(
                            ot[:], po[tc_i * 2 + ch][:],
                            bo_bc[:, ch * NQ:(ch + 1) * NQ], add)
                        nc.sync.dma_start(
                            out[tc_i * P:(tc_i + 1) * P,
                                ch * NQ:(ch + 1) * NQ], ot[:])

    nc.compile()
    return nc


def _get_nc():
    global _CACHED_NC
    if _CACHED_NC is None:
        _CACHED_NC = _build()
    return _CACHED_NC


def _arrange_wo(wo_np, g):
    """wo [D, D] -> per-core [ki=(l,hd), chunk=2m+pair, n].

    Chunk 2m+p holds rows for heads 4*(g^m) + 2p + {0,1} (m=0 local)."""
    wo_r = wo_np.reshape(H, HD, D)
    arr = np.empty((P, 8, D), dtype=np.float32)
    for m in range(4):
        for p in range(2):
            for l in range(2):
                head = 4 * (g ^ m) + 2 * p + l
                arr[l * HD:(l + 1) * HD, 2 * m + p, :] = wo_r[head]
    return arr


def kernel(x, wq, bq, wk, bk, wv, bv, wo, bo):
    from concourse.bass_utils import run_bass_kernel_spmd

    x = np.asarray(x, dtype=np.float32)
    wq = np.asarray(wq, dtype=np.float32)
    wk = np.asarray(wk, dtype=np.float32)
    wv = np.asarray(wv, dtype=np.float32)
    wo = np.asarray(wo, dtype=np.float32)
    bq = np.asarray(bq, dtype=np.float32)
    bk = np.asarray(bk, dtype=np.float32)
    bv = np.asarray(bv, dtype=np.float32)
    bo = np.asarray(bo, dtype=np.float32)

    nc = _get_nc()

    bf = ml_dtypes.bfloat16
    in_maps = []
    for c in range(N_CORES):
        b, g = c // G, c % G
        cs = slice(g * CW, (g + 1) * CW)
        xt = np.ascontiguousarray(x[b].T)
        xq = np.concatenate(
            [xt[:, (s ^ g) * NQ:((s ^ g) + 1) * NQ] for s in range(SQC)],
            axis=1)
        sel = np.zeros((P, 3, 2), dtype=np.float32)
        for m in (1, 2, 3):
            low = (g ^ m) < g
            sel[:, m - 1, 0 if low else 1] = 1.0
        in_maps.append({
            "xT": xt.astype(bf),
            "xqT": np.ascontiguousarray(xq).astype(bf),
            "wq": np.ascontiguousarray(wq[:, cs]).astype(bf),
            "wk": np.ascontiguousarray(wk[:, cs]).astype(bf),
            "wv": np.ascontiguousarray(wv[:, cs]).astype(bf),
            "bq": np.ascontiguousarray(bq[cs]),
            "bk": np.ascontiguousarray(bk[cs]),
            "bv": np.ascontiguousarray(bv[cs]),
            "wo": _arrange_wo(wo, g).astype(bf),
            "bo": bo,
            "sel": sel,
        })

    res = run_bass_kernel_spmd(nc, in_maps, core_ids=list(range(N_CORES)))

    full = np.empty((B, S, D), dtype=np.float32)
    for c in range(N_CORES):
        b, g = c // G, c % G
        full[b, g * NQ:(g + 1) * NQ, :] = res.results[c]["out"]
    return full


# revision 25
# speedup vs baseline: 1.4613x; 1.0480x over previous
"""Multi-head attention (B=2, S=2048, D=1024, H=16, HD=64) on 8 TRN2 cores.

Sharding (hybrid DP/TP, SPMD one-graph):
  core c: batch b = c//4, head-group g = c%4 (heads 4g..4g+3 of batch b).
  - QKV projections: Megatron column-split (each core its 4 heads), bf16.
  - queries are token-quarter XOR-permuted per core (local quarter s =
    true quarter s^g) via a separate host-permuted xqT input, so the
    exchange below is SPMD-uniform.
  - attention per (batch, head): QK^T row-packed 2 heads/pass
    (tile_position (0,0)/(64,0)); softmax exp split 40% ACT (exact) /
    60% DVE (Schraudolph int16-bitcast bf16); PV bf16 with a ones
    column producing the softmax denominator in row 64.
  - exchange: 6 pairwise (2-rank) AllGathers per core: round m pairs
    core g with g^m; both send their heads' att for the partner's true
    token quarter (local quarter m).  Wire: 3/4 of att, bf16.
  - O-projection: token-sharded — each core computes its true quarter
    (512 tokens) x full D with per-core-arranged wo; partner half of
    each AllGather selected with a per-core 0/1 input (DVE blend).
  - host gather: concat over (batch, token quarter).
"""

import numpy as np
import ml_dtypes

B, S, D = 2, 2048, 1024
H, HD = 16, 64
N_CORES = 8
G = 4                      # cores per batch group
HPC = 4                    # heads per core
CW = HPC * HD              # per-core projection width = 256
ATT_SCALE = float(HD) ** -0.5
P = 128
KC = D // P                # 8 contraction chunks
SC = S // P                # 16 key chunks of 128
NQ = 512                   # query chunk (= token quarter)
SQC = S // NQ              # 4 query chunks

LOG2E = 1.4426950408889634
# Schraudolph exp via int16 bitcast to bf16: bits = round(A16*x + B16)
A16 = 128.0 * LOG2E * ATT_SCALE
B16 = 128.0 * 127 - 7.33

# round-m 2-rank pairings (g <-> g^m) within each batch group
RG = {
    1: [[0, 1], [2, 3], [4, 5], [6, 7]],
    2: [[0, 2], [1, 3], [4, 6], [5, 7]],
    3: [[0, 3], [1, 2], [4, 7], [5, 6]],
}

_CACHED_NC = None


def _build():
    import concourse.mybir as mybir
    import concourse.tile as tile
    from concourse import bacc

    f32 = mybir.dt.float32
    f32r = mybir.dt.float32r
    bf16 = mybir.dt.bfloat16
    i16 = mybir.dt.int16
    Exp = mybir.ActivationFunctionType.Exp
    Ident = mybir.ActivationFunctionType.Identity
    add = mybir.AluOpType.add
    mult = mybir.AluOpType.mult

    nc = bacc.Bacc("TRN2", target_bir_lowering=False, debug=False,
                   num_devices=N_CORES)

    xT = nc.declare_dram_parameter("xT", [D, S], bf16, isOutput=False)
    xqT = nc.declare_dram_parameter("xqT", [D, S], bf16, isOutput=False)
    wq = nc.declare_dram_parameter("wq", [D, CW], bf16, isOutput=False)
    wk = nc.declare_dram_parameter("wk", [D, CW], bf16, isOutput=False)
    wv = nc.declare_dram_parameter("wv", [D, CW], bf16, isOutput=False)
    bq = nc.declare_dram_parameter("bq", [CW], f32, isOutput=False)
    bk = nc.declare_dram_parameter("bk", [CW], f32, isOutput=False)
    bv = nc.declare_dram_parameter("bv", [CW], f32, isOutput=False)
    # wo pre-arranged per core: [ki=(l,hd), chunk=2m+pair, n]
    wo = nc.declare_dram_parameter("wo", [P, 8, D], bf16, isOutput=False)
    bo = nc.declare_dram_parameter("bo", [D], f32, isOutput=False)
    sel = nc.declare_dram_parameter("sel", [P, 3, 2], f32, isOutput=False)
    out = nc.declare_dram_parameter("out", [NQ, D], f32, isOutput=True)

    with tile.TileContext(nc) as tc:
        with (
            tc.tile_pool(name="const", bufs=1) as const,
            tc.tile_pool(name="acts", bufs=1) as acts,
            tc.tile_pool(name="exps", bufs=4) as exps,
            tc.tile_pool(name="small", bufs=2) as small,
            tc.tile_pool(name="ostage", bufs=3) as ostage,
            tc.tile_pool(name="dram", bufs=1, space="DRAM") as dram,
        ):
            # ---- constant loads -------------------------------------
            wq_sb = const.tile([P, KC, CW], bf16, tag="wq")
            wk_sb = const.tile([P, KC, CW], bf16, tag="wk")
            wv_sb = const.tile([P, KC, CW], bf16, tag="wv")
            nc.sync.dma_start(wq_sb[:], wq.ap().rearrange("(k p) m -> p k m", p=P))
            nc.sync.dma_start(wk_sb[:], wk.ap().rearrange("(k p) m -> p k m", p=P))
            xqt_sb = const.tile([P, KC, S], bf16, tag="xq")
            xt_sb = const.tile([P, KC, S], bf16, tag="x")
            for ki in range(KC):
                nc.sync.dma_start(xqt_sb[:, ki, :], xqT[ki * P:(ki + 1) * P, :])
            for ki in range(KC):
                nc.sync.dma_start(xt_sb[:, ki, :], xT[ki * P:(ki + 1) * P, :])
            nc.sync.dma_start(wv_sb[:], wv.ap().rearrange("(k p) m -> p k m", p=P))
            wo_sb = const.tile([P, 8, D], bf16, tag="wo")
            nc.sync.dma_start(wo_sb[:], wo.ap())
            bq_sb = const.tile([P, 2], f32, tag="bq")
            bk_sb = const.tile([P, 2], f32, tag="bk")
            nc.sync.dma_start(bq_sb[:], bq.ap().rearrange("(j p) -> p j", p=P))
            nc.sync.dma_start(bk_sb[:], bk.ap().rearrange("(j p) -> p j", p=P))
            bv_bc = const.tile([P, CW], f32, tag="bv")
            bo_bc = const.tile([P, D], f32, tag="bo")
            nc.sync.dma_start(bv_bc[:], bv.ap().partition_broadcast(P))
            nc.sync.dma_start(bo_bc[:], bo.ap().partition_broadcast(P))
            sel_sb = const.tile([P, 3, 2], f32, tag="sel")
            nc.sync.dma_start(sel_sb[:], sel.ap())

            ones_f = const.tile([1, HD], f32, tag="onesf")
            ones_r = const.tile([1, HD], f32r, tag="onesr")
            nc.vector.memset(ones_f[:], 1.0)
            with nc.allow_low_precision("f32r is fp32 storage"):
                nc.vector.tensor_copy(ones_r[:], ones_f[:])

            qT_sb = acts.tile([P, 2, S], bf16, tag="qT")
            kT_sb = acts.tile([P, 2, S], bf16, tag="kT")
            # v per (mi, head): cols [v(64) | ones]
            v_sb = acts.tile([P, SC, HPC, HD + 1], bf16, tag="v")
            nc.vector.memset(v_sb[:, :, :, HD:HD + 1], 1.0)

            # ---- projections + attention, interleaved ---------------
            # stg[j2][sq] = my 2 heads' att for local quarter sq
            # (true quarter sq^g); sq=0 stays local, sq=m goes to
            # round-m partner g^m.
            stg = [dram.tile([SQC, P, NQ], bf16, tag=f"stg{p}",
                             name=f"stg{p}") for p in range(2)]
            agout = {}
            for j2 in range(2):
                for m in (1, 2, 3):
                    agout[(j2, m)] = dram.tile(
                        [2, P, NQ], bf16, tag=f"ago{j2}_{m}",
                        name=f"ago{j2}_{m}")
            with (
                tc.tile_pool(name="sc", bufs=3, space="PSUM") as scp,
                tc.tile_pool(name="pv", bufs=2, space="PSUM") as pvp,
            ):
                def qk_proj(j):
                    for (w_sb, b_sb, dst, src) in (
                            (wq_sb, bq_sb, qT_sb, xqt_sb),
                            (wk_sb, bk_sb, kT_sb, xt_sb)):
                        for si in range(SQC):
                            pst = scp.tile([P, 2, NQ], f32, tag="sc",
                                           name=f"pp{j}_{si}_{dst is kT_sb}")
                            ps = pst[:, 0, :]
                            for ki in range(KC):
                                nc.tensor.matmul(
                                    ps,
                                    w_sb[:, ki, j * P:(j + 1) * P],
                                    src[:, ki, si * NQ:(si + 1) * NQ],
                                    start=(ki == 0), stop=(ki == KC - 1),
                                )
                            # bias add on ACT (per-partition bias AP)
                            nc.scalar.activation(
                                dst[:, j, si * NQ:(si + 1) * NQ], ps,
                                Ident, bias=b_sb[:, j:j + 1], scale=1.0)

                def v_proj():
                    for si in range(SC):
                        pst = scp.tile([P, 2, NQ], f32, tag="sc",
                                       name=f"ppv{si}")
                        ps = pst[:, 0, :CW]
                        for ki in range(KC):
                            nc.tensor.matmul(
                                ps,
                                xt_sb[:, ki, si * P:(si + 1) * P],
                                wv_sb[:, ki, :],
                                start=(ki == 0), stop=(ki == KC - 1),
                            )
                        with nc.allow_low_precision("bf16 v"):
                            nc.vector.tensor_tensor(
                                v_sb[:, si, :, :HD],
                                ps.rearrange("p (h x) -> p h x", x=HD),
                                bv_bc.rearrange("p (h x) -> p h x", x=HD),
                                add)

                ucount = [0]

                def attention(j2):
                    for sq in (2, 3, 1, 0):      # slow rounds first
                        pvs = [pvp.tile([HD + 1, NQ], f32, tag="pv",
                                        name=f"pv{j2}_{sq}_{l}")
                               for l in range(2)]
                        for mi in range(SC):
                            sct = scp.tile([P, 2, NQ], f32, tag="sc",
                                           name=f"sc{j2}_{sq}_{mi}")
                            # QK row-packed: head l at rows 64*l, plane l
                            for l in range(2):
                                o = l * HD
                                nc.tensor.matmul(
                                    sct[:, l, :],
                                    kT_sb[o:o + HD, j2,
                                          mi * P:(mi + 1) * P],
                                    qT_sb[o:o + HD, j2,
                                          sq * NQ:(sq + 1) * NQ],
                                    start=True, stop=True,
                                )
                            # exp: ~69% ACT (exact), ~31% DVE Schraudolph
                            use_act = (ucount[0] % 13) not in (1, 4, 7, 10)
                            ucount[0] += 1
                            et = exps.tile([P, 2, NQ], bf16, tag="et")
                            if use_act:
                                nc.scalar.activation(
                                    et[:], sct[:], Exp, scale=ATT_SCALE)
                            else:
                                with nc.allow_low_precision(
                                        "schraudolph exp"):
                                    nc.vector.tensor_scalar(
                                        et[:].bitcast(i16), sct[:],
                                        A16, B16, mult, add)
                            for l in range(2):
                                h = 2 * j2 + l
                                nc.tensor.matmul(
                                    pvs[l][:],
                                    v_sb[:, mi, h, :],
                                    et[:, l, :],
                                    start=(mi == 0), stop=(mi == SC - 1),
                                )
                        # softmax divide, stage for exchange
                        for l in range(2):
                            den = small.tile([1, NQ], f32, tag="den")
                            nc.scalar.copy(den[:], pvs[l][HD:HD + 1, :])
                            rec = small.tile([1, NQ], f32, tag="rec")
                            rec_r = small.tile([1, NQ], f32r, tag="recr")
                            with nc.allow_low_precision("approx recip"):
                                nc.vector.reciprocal_approx_fast(
                                    rec[:], den[:])
                                nc.vector.tensor_copy(rec_r[:], rec[:])
                            rb = scp.tile([HD, NQ], f32, tag="sc",
                                          name=f"rb{j2}_{sq}_{l}")
                            nc.tensor.matmul(rb[:], ones_r[:], rec_r[:],
                                             start=True, stop=True)
                            rb_sb = small.tile([HD, NQ], f32, tag="rbs")
                            nc.scalar.copy(rb_sb[:], rb[:])
                            at = small.tile([HD, NQ], bf16, tag="at")
                            with nc.allow_low_precision("bf16 att"):
                                nc.vector.tensor_tensor(
                                    at[:], pvs[l][:HD, :], rb_sb[:], mult)
                            nc.sync.dma_start(
                                stg[j2][sq, l * HD:(l + 1) * HD, :], at[:])
                        if sq in (1, 2, 3):
                            nc.gpsimd.collective_compute(
                                "AllGather", mybir.AluOpType.bypass,
                                replica_groups=RG[sq],
                                ins=[stg[j2][sq, :, :]],
                                outs=[agout[(j2, sq)].opt()],
                            )

                qk_proj(0)
                v_proj()
                attention(0)
                qk_proj(1)
                attention(1)

            # ---- O-projection (token-sharded, full width) -----------
            with (
                tc.tile_pool(name="attk", bufs=3) as attk,
                tc.tile_pool(name="op", bufs=8, space="PSUM") as op,
            ):
                po = [op.tile([P, NQ], f32, tag="po", name=f"po{i}")
                      for i in range(8)]

                def oproj_chunk(c, atk):
                    for tc_i in range(SQC):
                        for ch in range(2):
                            nc.tensor.matmul(
                                po[tc_i * 2 + ch][:],
                                atk[:, tc_i * P:(tc_i + 1) * P],
                                wo_sb[:, c, ch * NQ:(ch + 1) * NQ],
                                start=(c == 0), stop=(c == 7),
                            )

                for m in range(4):      # 0 = local, 1..3 = rounds
                    for j2 in range(2):
                        c = 2 * m + j2
                        if m == 0:
                            atk = attk.tile([P, NQ], bf16, tag="atk",
                                            name=f"atk{c}")
                            nc.sync.dma_start(atk[:], stg[j2][0, :, :])
                        else:
                            a2 = attk.tile([P, 2, NQ], bf16, tag="a2",
                                           name=f"a2_{c}")
                            nc.sync.dma_start(
                                a2[:],
                                agout[(j2, m)][:, :, :]
                                .rearrange("h r s -> r h s"))
                            t0 = attk.tile([P, NQ], bf16, tag="t0",
                                           name=f"t0_{c}")
                            atk = attk.tile([P, NQ], bf16, tag="atk",
                                            name=f"atk{c}")
                            with nc.allow_low_precision("half select"):
                                nc.vector.tensor_scalar(
                                    t0[:], a2[:, 0, :],
                                    sel_sb[:, m - 1, 0:1], None, mult)
                                nc.vector.tensor_scalar(
                                    atk[:], a2[:, 1, :],
                                    sel_sb[:, m - 1, 1:2], None, mult)
                                nc.vector.tensor_tensor(
                                    atk[:], atk[:], t0[:], add)
                        oproj_chunk(c, atk)

                for tc_i in range(SQC):
                    for ch in range(2):
                        ot = ostage.tile([P, NQ], f32, tag="ot")
                        nc.vector.tensor_tensor(
                            ot[:], po[tc_i * 2 + ch][:],
                            bo_bc[:, ch * NQ:(ch + 1) * NQ], add)
                        nc.sync.dma_start(
                            out[tc_i * P:(tc_i + 1) * P,
                                ch * NQ:(ch + 1) * NQ], ot[:])

    nc.compile()
    return nc


def _get_nc():
    global _CACHED_NC
    if _CACHED_NC is None:
        _CACHED_NC = _build()
    return _CACHED_NC


def _arrange_wo(wo_np, g):
    """wo [D, D] -> per-core [ki=(l,hd), chunk=2m+pair, n].

    Chunk 2m+p holds rows for heads 4*(g^m) + 2p + {0,1} (m=0 local)."""
    wo_r = wo_np.reshape(H, HD, D)
    arr = np.empty((P, 8, D), dtype=np.float32)
    for m in range(4):
        for p in range(2):
            for l in range(2):
                head = 4 * (g ^ m) + 2 * p + l
                arr[l * HD:(l + 1) * HD, 2 * m + p, :] = wo_r[head]
    return arr


def kernel(x, wq, bq, wk, bk, wv, bv, wo, bo):
    from concourse.bass_utils import run_bass_kernel_spmd

    x = np.asarray(x, dtype=np.float32)
    wq = np.asarray(wq, dtype=np.float32)
    wk = np.asarray(wk, dtype=np.float32)
    wv = np.asarray(wv, dtype=np.float32)
    wo = np.asarray(wo, dtype=np.float32)
    bq = np.asarray(bq, dtype=np.float32)
    bk = np.asarray(bk, dtype=np.float32)
    bv = np.asarray(bv, dtype=np.float32)
    bo = np.asarray(bo, dtype=np.float32)

    nc = _get_nc()

    bf = ml_dtypes.bfloat16
    in_maps = []
    for c in range(N_CORES):
        b, g = c // G, c % G
        cs = slice(g * CW, (g + 1) * CW)
        xt = np.ascontiguousarray(x[b].T)
        xq = np.concatenate(
            [xt[:, (s ^ g) * NQ:((s ^ g) + 1) * NQ] for s in range(SQC)],
            axis=1)
        sel = np.zeros((P, 3, 2), dtype=np.float32)
        for m in (1, 2, 3):
            low = (g ^ m) < g
            sel[:, m - 1, 0 if low else 1] = 1.0
        in_maps.append({
            "xT": xt.astype(bf),
            "xqT": np.ascontiguousarray(xq).astype(bf),
            "wq": np.ascontiguousarray(wq[:, cs]).astype(bf),
            "wk": np.ascontiguousarray(wk[:, cs]).astype(bf),
            "wv": np.ascontiguousarray(wv[:, cs]).astype(bf),
            "bq": np.ascontiguousarray(bq[cs]),
            "bk": np.ascontiguousarray(bk[cs]),
            "bv": np.ascontiguousarray(bv[cs]),
            "wo": _arrange_wo(wo, g).astype(bf),
            "bo": bo,
            "sel": sel,
        })

    res = run_bass_kernel_spmd(nc, in_maps, core_ids=list(range(N_CORES)))

    full = np.empty((B, S, D), dtype=np.float32)
    for c in range(N_CORES):
        b, g = c // G, c % G
        full[b, g * NQ:(g + 1) * NQ, :] = res.results[c]["out"]
    return full
